# revision 1
# baseline (speedup 1.0000x reference)
"""Longformer-with-motifs encoder on 8 trn2 NeuronCores.

Sharding: batch(2 groups of 4 cores) x Megatron tensor-parallel(4: 3 heads
each, FF/4) with 2 bf16 AllReduces per layer inside each 4-core group.

Device layout: activations feature-major xT [768, S] as SBUF tiles
[128, 6*1024].  GEMMs in bf16 (fp32 PSUM accumulation).  Softmax is
max-free (scores are small by construction; masked entries multiplied by
a 0/1 mask post-exp, CLS column handled separately so exp(-1e9)==0
semantics are preserved exactly).  Attention computed transposed
(scoresT[j,i]) so no probability transposes are needed; per-query softmax
sums come from an interleaved ones-column in the PV stationary operand.
LayerNorm: stats via PE ones-matmuls, rsqrt row broadcast via gpsimd
partition_broadcast, apply via gpsimd scalar_tensor_tensor, scale/bias via
ACT Identity(in*s+b).  Residual stream kept as x/4 in f32 so the residual
add folds into the PSUM eviction before each AllReduce.
"""

import sys

sys.path.insert(0, "/opt/trn_rl_repo")

import numpy as np
import ml_dtypes

import concourse.bacc as bacc
import concourse.bass as bass
import concourse.tile as tile
import concourse.mybir as mybir
from concourse.bass_utils import run_bass_kernel_spmd

BF16 = mybir.dt.bfloat16
F32 = mybir.dt.float32
bf16 = ml_dtypes.bfloat16

B, S, L, H, D, FF, V = 2, 1024, 12, 12, 768, 3072, 50265
DH = D // H
W1 = 256
MAXPOS = 4098
EPS = 1e-5
N_CORES = 8
TP = 4                      # tensor-parallel degree within a group
HC = H // TP                # heads per core = 3
HD = HC * DH                # 192 local head dims
FFC = FF // TP              # 768 local ff dims
KT = D // 128               # 6 k-tiles over feature dim
NSP = 2                     # two 512-token spans
SPW = 512

mm = None  # set in build


def _jts(sp):
    return list(range(0, 6)) if sp == 0 else list(range(2, 8))


def build_program():
    nc = bacc.Bacc("TRN2", target_bir_lowering=False, debug=False,
                   num_devices=N_CORES)

    def din(name, shape, dt=BF16):
        return nc.dram_tensor(name, shape, dt, kind="ExternalInput").ap()

    x0T_d = din("x0T", [D, S])
    wqkkg_d = din("wqkkg", [L, D + 1, 3 * HD])
    wvvg_d = din("wvvg", [L, D + 1, 2 * HD])
    wo_d = din("wo", [L, HD + 1, D])
    wqg_d = din("wqg", [L, D + 1, HD])
    wi_d = din("wi", [L, D + 1, FFC])
    wo2_d = din("wo2", [L, FFC + 1, D])
    lnc_d = din("lnc", [L + 1, D, 4], dt=F32)
    mask_d = din("mask", [12, 128, SPW])
    motif_d = din("motif", [415, 1])
    wd_d = din("wd", [1183, D])
    wp_d = din("wp", [D + 1, 2])
    logits_d = nc.dram_tensor("logits", [2, 1], F32, kind="ExternalOutput").ap()

    ACT = mybir.ActivationFunctionType
    ALU = mybir.AluOpType

    with tile.TileContext(nc) as tc:
        with tc.tile_pool(name="sb1", bufs=1) as p1, \
             tc.tile_pool(name="sb2", bufs=2) as p2, \
             tc.tile_pool(name="sb3", bufs=3) as p3, \
             tc.tile_pool(name="psA", bufs=3, space="PSUM") as psA, \
             tc.tile_pool(name="psS", bufs=2, space="PSUM") as psS, \
             tc.tile_pool(name="psO", bufs=1, space="PSUM") as psO, \
             tc.tile_pool(name="psR", bufs=2, space="PSUM") as psR, \
             tc.tile_pool(name="dram", bufs=2, space="DRAM") as dpool:

            # ---------------- persistent constants ----------------
            ones_row = p1.tile([1, S], BF16, tag="ones_row")
            nc.vector.memset(ones_row[:], 1.0)
            ones128 = p1.tile([128, 1], BF16, tag="ones128")
            nc.vector.memset(ones128[:], 1.0)
            ones128f = p1.tile([128, 1], F32, tag="ones128f")
            nc.vector.memset(ones128f[:], 1.0)
            eps_t = p1.tile([1, 1], F32, tag="eps_t")
            nc.vector.memset(eps_t[:], EPS)
            zero_t = p1.tile([128, 1], F32, tag="zero_t")
            nc.vector.memset(zero_t[:], 0.0)
            mask_s = p1.tile([128, 12 * SPW], BF16, tag="mask_s")
            for i in range(12):
                nc.sync.dma_start(mask_s[:, i * SPW:(i + 1) * SPW], mask_d[i])

            # activations (persistent tags)
            xb = p1.tile([128, KT * S], BF16, tag="xb")       # bf16 x (GEMM in)
            q_s = p1.tile([64, HC * S], BF16, tag="q_s")
            k_s = p1.tile([64, HC * S], BF16, tag="k_s")
            kg_s = p1.tile([64, HC * S], BF16, tag="kg_s")
            vvg_s = p1.tile([128, 8 * (HC * 65 + HD)], BF16, tag="vvg_s")
            VBLK = HC * 65 + HD                                # 387
            for tt in range(8):
                for h in range(HC):
                    nc.vector.memset(vvg_s[:, tt * VBLK + 65 * h + 64:
                                           tt * VBLK + 65 * h + 65], 1.0)
            att0 = p1.tile([128, S], BF16, tag="att0")         # heads 0,1
            att1 = p1.tile([65, S], BF16, tag="att1")          # head 2 + ones
            nc.vector.memset(att1[64:65, :], 1.0)
            qg_s = p1.tile([64, HC], BF16, tag="qg_s")
            hb = p1.tile([128, KT * S], BF16, tag="hb")        # ffn hidden

            def wtile(tag, cols, bufs=1):
                pool = p1 if bufs == 1 else p2
                return pool.tile([128, cols], BF16, tag=tag, name=tag)

            # ---------------- helpers ----------------
            def dma_w(t, src, n_k, m, last_rows):
                """load [n_k*128(+last) , m] weight into [128, n_k_tot*m] tile"""
                for kt in range(n_k):
                    nc.sync.dma_start(t[:, kt * m:(kt + 1) * m],
                                      src[kt * 128:(kt + 1) * 128, :])
                if last_rows:
                    nc.sync.dma_start(t[0:last_rows, n_k * m:(n_k + 1) * m],
                                      src[n_k * 128:n_k * 128 + last_rows, :])

            def layer_norm(z_t, lnc_t, c0, out_xq=True):
                """z_t: [128, KT*S] bf16 -> writes xb (+ xq).  lnc cols c0,c0+1."""
                u_row = p2.tile([1, S], BF16, tag="u_row")
                w_row = p2.tile([1, S], BF16, tag="w_row")
                for sp in range(NSP):
                    mp = psR.tile([1, SPW], F32, tag="row")
                    mq = psR.tile([1, SPW], F32, tag="row")
                    for kt in range(KT):
                        zsl = z_t[:, kt * S + sp * SPW: kt * S + (sp + 1) * SPW]
                        zsq = p2.tile([128, SPW], BF16, tag="zsq")
                        nc.scalar.activation(zsq[:], zsl, ACT.Square, bias=zero_t[:])
                        nc.tensor.matmul(mp[:], lhsT=ones128[:], rhs=zsl,
                                         start=(kt == 0), stop=(kt == KT - 1))
                        nc.tensor.matmul(mq[:], lhsT=ones128[:], rhs=zsq[:],
                                         start=(kt == 0), stop=(kt == KT - 1))
                    m_s = p2.tile([1, SPW], F32, tag="m_s")
                    nc.scalar.activation(m_s[:], mp[:], ACT.Copy, scale=1.0 / D)
                    m2 = p2.tile([1, SPW], F32, tag="m2")
                    nc.scalar.activation(m2[:], m_s[:], ACT.Square, bias=zero_t[0:1, :])
                    var = p2.tile([1, SPW], F32, tag="var")
                    nc.vector.scalar_tensor_tensor(
                        var[:], mq[:], 1.0 / D, m2[:], ALU.mult, ALU.subtract)
                    std = p2.tile([1, SPW], F32, tag="std")
                    nc.scalar.activation(std[:], var[:], ACT.Sqrt,
                                         bias=eps_t[:])
                    usl = u_row[0:1, sp * SPW:(sp + 1) * SPW]
                    with nc.allow_low_precision(reason="bf16 rsqrt row"):
                        nc.vector.reciprocal(usl, std[:])
                    nc.vector.scalar_tensor_tensor(
                        w_row[0:1, sp * SPW:(sp + 1) * SPW],
                        m_s[:], 1.0, usl, ALU.mult, ALU.mult)
                U0 = p1.tile([128, S], BF16, tag="U0", name="U0")
                W0 = p1.tile([128, S], BF16, tag="W0", name="W0")
                for sp in range(NSP):
                    nc.gpsimd.partition_broadcast(
                        U0[:, sp * SPW:(sp + 1) * SPW],
                        u_row[0:1, sp * SPW:(sp + 1) * SPW])
                    nc.gpsimd.partition_broadcast(
                        W0[:, sp * SPW:(sp + 1) * SPW],
                        w_row[0:1, sp * SPW:(sp + 1) * SPW])
                for kt in range(KT):
                    s_col = lnc_t[:, 4 * kt + c0: 4 * kt + c0 + 1]
                    b_col = lnc_t[:, 4 * kt + c0 + 1: 4 * kt + c0 + 2]
                    for sp in range(NSP):
                        zsl = z_t[:, kt * S + sp * SPW: kt * S + (sp + 1) * SPW]
                        t1 = p2.tile([128, SPW], F32, tag="t1")
                        nc.vector.scalar_tensor_tensor(
                            t1[:], zsl, 1.0, U0[:, sp * SPW:(sp + 1) * SPW],
                            ALU.mult, ALU.mult)
                        u2 = p2.tile([128, SPW], F32, tag="u2")
                        nc.vector.scalar_tensor_tensor(
                            u2[:], t1[:], 1.0, W0[:, sp * SPW:(sp + 1) * SPW],
                            ALU.mult, ALU.subtract)
                        xbs = xb[:, kt * S + sp * SPW: kt * S + (sp + 1) * SPW]
                        nc.scalar.activation(xbs, u2[:], ACT.Identity,
                                             bias=b_col, scale=s_col)

            def allreduce_z(z_loc):
                bi = dpool.tile([128, KT * S], BF16, name="ar_in")
                bo = dpool.tile([128, KT * S], BF16, name="ar_out")
                nc.sync.dma_start(bi[:], z_loc[:])
                nc.gpsimd.collective_compute(
                    "AllReduce", ALU.add,
                    replica_groups=[[0, 1, 2, 3], [4, 5, 6, 7]],
                    ins=[bi[:].opt()], outs=[bo[:].opt()])
                z_new = p1.tile([128, KT * S], BF16, tag="z", name="z_new")
                nc.sync.dma_start(z_new[:], bo[:])
                return z_new

            # ---------------- embeddings ----------------
            z0 = p1.tile([128, KT * S], BF16, tag="z", name="z0")
            for kt in range(KT):
                nc.sync.dma_start(z0[:, kt * S:(kt + 1) * S],
                                  x0T_d[kt * 128:(kt + 1) * 128, :])
            lnc_e = p2.tile([128, 4 * KT], F32, tag="lnc")
            for kt in range(KT):
                nc.sync.dma_start(lnc_e[:, 4 * kt:4 * kt + 4],
                                  lnc_d[L, kt * 128:(kt + 1) * 128, :])
            layer_norm(z0, lnc_e, 0)

            # ---------------- layers ----------------
            for l in range(L):
                wqkkg = wtile("wqkkg", 7 * 3 * HD)
                dma_w(wqkkg, wqkkg_d[l], KT, 3 * HD, 1)
                wvvg = wtile("wvvg", 7 * 2 * HD)
                dma_w(wvvg, wvvg_d[l], KT, 2 * HD, 1)
                wqg = wtile("wqg", 7 * HD)
                dma_w(wqg, wqg_d[l], KT, HD, 1)
                wo_s = wtile("wo_s", 2 * D)
                nc.sync.dma_start(wo_s[:, 0:D], wo_d[l, 0:128, :])
                nc.sync.dma_start(wo_s[0:65, D:2 * D], wo_d[l, 128:193, :])
                lnc_t = p2.tile([128, 4 * KT], F32, tag="lnc")
                for kt in range(KT):
                    nc.sync.dma_start(lnc_t[:, 4 * kt:4 * kt + 4],
                                      lnc_d[l, kt * 128:(kt + 1) * 128, :])
                wi_s = wtile("wi_s", 7 * FFC)
                dma_w(wi_s, wi_d[l], KT, FFC, 1)
                wo2_s = wtile("wo2_s", 7 * D)
                dma_w(wo2_s, wo2_d[l], KT, D, 1)

                # ---- qkv/kg projections: out[64m, tok] ----
                for sp in range(NSP):
                    for mt in range(5):
                        mw = 128 if mt < 4 else 64
                        ps = psA.tile([128, SPW], F32, tag="psA")
                        for kt in range(KT + 1):
                            kk = 128 if kt < KT else 1
                            lhsT = wqkkg[0:kk, kt * 3 * HD + mt * 128:
                                         kt * 3 * HD + mt * 128 + mw]
                            rhs = (xb[:, kt * S + sp * SPW:kt * S + (sp + 1) * SPW]
                                   if kt < KT else
                                   ones_row[0:1, sp * SPW:(sp + 1) * SPW])
                            nc.tensor.matmul(ps[0:mw, :], lhsT=lhsT, rhs=rhs,
                                             start=(kt == 0), stop=(kt == KT))
                        for sub in range(2 if mt < 4 else 1):
                            m = 2 * mt + sub
                            kind, h = m // 3, m % 3
                            dest = (q_s, k_s, kg_s)[kind]
                            nc.vector.tensor_copy(
                                dest[0:64, h * S + sp * SPW: h * S + (sp + 1) * SPW],
                                ps[64 * sub:64 * sub + 64, :])

                # ---- v/vg projections: out[tok, dh] ----
                for tt in range(8):
                    ps = psA.tile([128, 2 * HD], F32, tag="psA")
                    for kt in range(KT + 1):
                        kk = 128 if kt < KT else 1
                        lhsT = (xb[:, kt * S + tt * 128: kt * S + (tt + 1) * 128]
                                if kt < KT else ones_row[0:1, 0:128])
                        rhs = wvvg[0:kk, kt * 2 * HD:(kt + 1) * 2 * HD]
                        nc.tensor.matmul(ps[:], lhsT=lhsT, rhs=rhs,
                                         start=(kt == 0), stop=(kt == KT))
                    base = tt * VBLK
                    for h in range(HC):
                        nc.vector.tensor_copy(
                            vvg_s[:, base + 65 * h: base + 65 * h + 64],
                            ps[:, 64 * h:64 * h + 64])
                    nc.vector.tensor_copy(
                        vvg_s[:, base + 65 * HC: base + 65 * HC + HD],
                        ps[:, HD:2 * HD])

                # ---- global query projection qgT [192, 1] ----
                for mt in range(2):
                    mw = 128 if mt == 0 else 64
                    ps = psR.tile([128, 1], F32, tag="row")
                    for kt in range(KT + 1):
                        kk = 128 if kt < KT else 1
                        lhsT = wqg[0:kk, kt * HD + mt * 128: kt * HD + mt * 128 + mw]
                        rhs = (xb[:, kt * S: kt * S + 1] if kt < KT
                               else ones_row[0:1, 0:1])
                        nc.tensor.matmul(ps[0:mw, :], lhsT=lhsT, rhs=rhs,
                                         start=(kt == 0), stop=(kt == KT))
                    for sub in range(2 if mt == 0 else 1):
                        h = 2 * mt + sub
                        nc.vector.tensor_copy(qg_s[0:64, h:h + 1],
                                              ps[64 * sub:64 * sub + 64, :])

                # ---- attention ----
                for h in range(HC):
                    # global attention for this head -> og_ps [65,1]
                    sg = psS.tile([128, 8], F32, tag="sc")
                    for jt in range(8):
                        nc.tensor.matmul(
                            sg[:, jt:jt + 1],
                            lhsT=kg_s[0:64, h * S + jt * 128: h * S + (jt + 1) * 128],
                            rhs=qg_s[0:64, h:h + 1], start=True, stop=True)
                    esg = p2.tile([128, 8], BF16, tag="esg")
                    acc = p2.tile([128, 1], F32, tag="acc_sg")
                    nc.scalar.activation(esg[:], sg[:], ACT.Exp, bias=zero_t[:], accum_out=acc[:])
                    og = psR.tile([65, 1], F32, tag="row")
                    nc.tensor.matmul(og[64:65, :], lhsT=ones128f[:], rhs=acc[:],
                                     start=True, stop=True)
                    for jt in range(8):
                        nc.tensor.matmul(
                            og[0:64, :],
                            lhsT=vvg_s[:, jt * VBLK + 65 * HC + 64 * h:
                                       jt * VBLK + 65 * HC + 64 * h + 64],
                            rhs=esg[:, jt:jt + 1],
                            start=(jt == 0), stop=(jt == 7))
                    for sp in range(NSP):
                        outT = psO.tile([65, SPW], F32, tag="outT")
                        jts = _jts(sp)
                        for jj, jt in enumerate(jts):
                            sc = psS.tile([128, SPW], F32, tag="sc")
                            nc.tensor.matmul(
                                sc[:],
                                lhsT=k_s[0:64, h * S + jt * 128: h * S + (jt + 1) * 128],
                                rhs=q_s[0:64, h * S + sp * SPW: h * S + (sp + 1) * SPW],
                                start=True, stop=True)
                            ex = p3.tile([128, SPW], BF16, tag="ex")
                            nc.scalar.activation(ex[:], sc[:], ACT.Exp, bias=zero_t[:])
                            exm = p3.tile([128, SPW], BF16, tag="exm")
                            midx = 6 * sp + jj
                            nc.vector.scalar_tensor_tensor(
                                exm[:], ex[:], 1.0,
                                mask_s[:, midx * SPW:(midx + 1) * SPW],
                                ALU.mult, ALU.mult)
                            nc.tensor.matmul(
                                outT[:],
                                lhsT=vvg_s[:, jt * VBLK + 65 * h: jt * VBLK + 65 * h + 65],
                                rhs=exm[:], start=(jj == 0), stop=False)
                        # CLS column (key 0) for all queries
                        csc = psR.tile([1, SPW], F32, tag="row")
                        nc.tensor.matmul(
                            csc[:], lhsT=k_s[0:64, h * S: h * S + 1],
                            rhs=q_s[0:64, h * S + sp * SPW: h * S + (sp + 1) * SPW],
                            start=True, stop=True)
                        cex = p2.tile([1, SPW], BF16, tag="cex")
                        nc.scalar.activation(cex[:], csc[:], ACT.Exp, bias=zero_t[0:1, :])
                        nc.tensor.matmul(outT[:],
                                         lhsT=vvg_s[0:1, 65 * h: 65 * h + 65],
                                         rhs=cex[:], start=False, stop=True)
                        if sp == 0:
                            # overwrite CLS token output with global attention
                            nc.vector.tensor_copy(outT[0:65, 0:1], og[0:65, :])
                        # normalize by the sums row and store
                        rr = p2.tile([1, SPW], BF16, tag="rr")
                        with nc.allow_low_precision(reason="bf16 softmax recip"):
                            nc.vector.reciprocal(rr[:], outT[64:65, :])
                        rb = p2.tile([64, SPW], BF16, tag="rb")
                        nc.gpsimd.partition_broadcast(rb[:], rr[:])
                        dest = (att0[64 * h:64 * h + 64,
                                     sp * SPW:(sp + 1) * SPW] if h < 2 else
                                att1[0:64, sp * SPW:(sp + 1) * SPW])
                        nc.vector.scalar_tensor_tensor(
                            dest, outT[0:64, :], 1.0, rb[:], ALU.mult, ALU.mult)

                # ---- output projection + residual ----
                z_loc = p1.tile([128, KT * S], BF16, tag="z_loc", name="z_loc")
                for sp in range(NSP):
                    for mt in range(KT):
                        ps = psA.tile([128, SPW], F32, tag="psA")
                        nc.tensor.matmul(
                            ps[:], lhsT=wo_s[:, mt * 128:(mt + 1) * 128],
                            rhs=att0[:, sp * SPW:(sp + 1) * SPW],
                            start=True, stop=False)
                        nc.tensor.matmul(
                            ps[:], lhsT=wo_s[0:65, D + mt * 128: D + (mt + 1) * 128],
                            rhs=att1[:, sp * SPW:(sp + 1) * SPW],
                            start=False, stop=True)
                        nc.vector.scalar_tensor_tensor(
                            z_loc[:, mt * S + sp * SPW: mt * S + (sp + 1) * SPW],
                            xb[:, mt * S + sp * SPW: mt * S + (sp + 1) * SPW],
                            0.25, ps[:], ALU.mult, ALU.add)
                z1 = allreduce_z(z_loc)
                layer_norm(z1, lnc_t, 0)

                # ---- FFN ----
                for sp in range(NSP):
                    for mt in range(KT):
                        ps = psA.tile([128, SPW], F32, tag="psA")
                        for kt in range(KT + 1):
                            kk = 128 if kt < KT else 1
                            lhsT = wi_s[0:kk, kt * FFC + mt * 128:
                                        kt * FFC + (mt + 1) * 128]
                            rhs = (xb[:, kt * S + sp * SPW: kt * S + (sp + 1) * SPW]
                                   if kt < KT else
                                   ones_row[0:1, sp * SPW:(sp + 1) * SPW])
                            nc.tensor.matmul(ps[:], lhsT=lhsT, rhs=rhs,
                                             start=(kt == 0), stop=(kt == KT))
                        nc.scalar.activation(
                            hb[:, mt * S + sp * SPW: mt * S + (sp + 1) * SPW],
                            ps[:], ACT.Gelu, bias=zero_t[:])
                z_loc2 = p1.tile([128, KT * S], BF16, tag="z_loc", name="z_loc2")
                for sp in range(NSP):
                    for mt in range(KT):
                        ps = psA.tile([128, SPW], F32, tag="psA")
                        for kt in range(KT + 1):
                            kk = 128 if kt < KT else 1
                            lhsT = wo2_s[0:kk, kt * D + mt * 128:
                                         kt * D + (mt + 1) * 128]
                            rhs = (hb[:, kt * S + sp * SPW: kt * S + (sp + 1) * SPW]
                                   if kt < KT else
                                   ones_row[0:1, sp * SPW:(sp + 1) * SPW])
                            nc.tensor.matmul(ps[:], lhsT=lhsT, rhs=rhs,
                                             start=(kt == 0), stop=(kt == KT))
                        nc.vector.scalar_tensor_tensor(
                            z_loc2[:, mt * S + sp * SPW: mt * S + (sp + 1) * SPW],
                            xb[:, mt * S + sp * SPW: mt * S + (sp + 1) * SPW],
                            0.25, ps[:], ALU.mult, ALU.add)
                z2 = allreduce_z(z_loc2)
                layer_norm(z2, lnc_t, 2, out_xq=(l < L - 1))

            # ---------------- classification head ----------------
            wd_s = p1.tile([128, 10 * D], BF16, tag="wd_s")
            for kt in range(9):
                nc.sync.dma_start(wd_s[:, kt * D:(kt + 1) * D],
                                  wd_d[kt * 128:(kt + 1) * 128, :])
            nc.sync.dma_start(wd_s[0:31, 9 * D:10 * D], wd_d[9 * 128:1183, :])
            wp_s = p1.tile([128, 14], BF16, tag="wp_s")
            for kt in range(6):
                nc.sync.dma_start(wp_s[:, 2 * kt:2 * kt + 2],
                                  wp_d[kt * 128:(kt + 1) * 128, :])
            nc.sync.dma_start(wp_s[0:1, 12:14], wp_d[768:769, :])
            mot_s = p1.tile([128, 4], BF16, tag="mot_s")
            for c in range(4):
                sz = 128 if c < 3 else 31
                nc.sync.dma_start(mot_s[0:sz, c:c + 1],
                                  motif_d[128 * c:128 * c + sz, :])

            ty = p1.tile([128, KT], BF16, tag="ty")
            for mt in range(KT):
                ps = psR.tile([128, 1], F32, tag="row")
                for kt in range(10):
                    kk = 128 if kt < 9 else 31
                    lhsT = wd_s[0:kk, kt * D + mt * 128: kt * D + (mt + 1) * 128]
                    rhs = (xb[:, kt * S: kt * S + 1] if kt < KT
                           else mot_s[0:kk, kt - KT: kt - KT + 1])
                    nc.tensor.matmul(ps[:], lhsT=lhsT, rhs=rhs,
                                     start=(kt == 0), stop=(kt == 9))
                nc.scalar.activation(ty[:, mt:mt + 1], ps[:], ACT.Tanh, bias=zero_t[:])
            lg_ps = psR.tile([2, 1], F32, tag="row")
            for kt in range(7):
                kk = 128 if kt < 6 else 1
                lhsT = wp_s[0:kk, 2 * kt:2 * kt + 2]
                rhs = ty[:, kt:kt + 1] if kt < 6 else ones_row[0:1, 0:1]
                nc.tensor.matmul(lg_ps[:], lhsT=lhsT, rhs=rhs,
                                 start=(kt == 0), stop=(kt == 6))
            lg_s = p1.tile([2, 1], F32, tag="lg_s")
            nc.vector.tensor_copy(lg_s[:], lg_ps[:])
            nc.sync.dma_start(logits_d[:], lg_s[:])

    nc.compile()
    return nc


def prep_inputs(inputs):
    """host-side sharding: returns in_maps for the 8 cores"""
    f32 = np.float32
    ids = np.asarray(inputs["input_ids"])
    motif = np.asarray(inputs["motif_dist"], f32)
    emb_word = np.asarray(inputs["emb_word"], f32)
    emb_pos = np.asarray(inputs["emb_pos"], f32)
    emb_type = np.asarray(inputs["emb_type"], f32)
    g = {k: np.asarray(inputs[k], f32) for k in
         ("Wq", "bq", "Wk", "bk", "Wv", "bv", "Wqg", "bqg", "Wkg", "bkg",
          "Wvg", "bvg", "Wo", "bo", "ln1_s", "ln1_b", "Wi", "bi", "Wo2",
          "bo2", "ln2_s", "ln2_b", "emb_ln_s", "emb_ln_b",
          "head_Wd", "head_bd", "head_Wp", "head_bp")}
    scale = 1.0 / np.sqrt(DH)

    # masks [12, 128, 512]
    mask = np.zeros((12, 128, SPW), f32)
    for sp in range(NSP):
        for jj, jt in enumerate(_jts(sp)):
            j = 128 * jt + np.arange(128)[:, None]
            i = SPW * sp + np.arange(SPW)[None, :]
            mask[6 * sp + jj] = ((np.abs(j - i) <= W1) & (j != 0)).astype(f32)

    # lnc [13, 768, 4] : (s1, b1, s2, b2) per layer; [12] = embedding LN
    lnc = np.zeros((L + 1, D, 4), f32)
    for l in range(L):
        lnc[l, :, 0] = g["ln1_s"][l]
        lnc[l, :, 1] = g["ln1_b"][l]
        lnc[l, :, 2] = g["ln2_s"][l]
        lnc[l, :, 3] = g["ln2_b"][l]
    lnc[L, :, 0] = g["emb_ln_s"]
    lnc[L, :, 1] = g["emb_ln_b"]

    wd_aug = np.concatenate([g["head_Wd"], g["head_bd"][None, :]], 0)  # [1183,768]
    wp_aug = np.concatenate([g["head_Wp"], g["head_bp"][None, :]], 0)  # [769,2]

    in_maps = []
    for core in range(N_CORES):
        b, r = core // TP, core % TP
        hs = slice(HD * r, HD * (r + 1))
        fs = slice(FFC * r, FFC * (r + 1))
        x0 = emb_word[ids[b]] + emb_pos[2:2 + S] + emb_type[0]
        d = {"x0T": x0.T.copy()}
        wqkkg = np.zeros((L, D + 1, 3 * HD), f32)
        wvvg = np.zeros((L, D + 1, 2 * HD), f32)
        wo = np.zeros((L, HD + 1, D), f32)
        wqg = np.zeros((L, D + 1, HD), f32)
        wi = np.zeros((L, D + 1, FFC), f32)
        wo2 = np.zeros((L, FFC + 1, D), f32)
        for l in range(L):
            wqkkg[l, :D] = np.concatenate(
                [g["Wq"][l][:, hs] * scale, g["Wk"][l][:, hs],
                 g["Wkg"][l][:, hs]], 1)
            wqkkg[l, D] = np.concatenate(
                [g["bq"][l][hs] * scale, g["bk"][l][hs], g["bkg"][l][hs]])
            wvvg[l, :D] = np.concatenate(
                [g["Wv"][l][:, hs], g["Wvg"][l][:, hs]], 1)
            wvvg[l, D] = np.concatenate([g["bv"][l][hs], g["bvg"][l][hs]])
            wo[l, :HD] = g["Wo"][l][hs, :]
            wo[l, HD] = g["bo"][l] * 0.25
            wqg[l, :D] = g["Wqg"][l][:, hs] * scale
            wqg[l, D] = g["bqg"][l][hs] * scale
            wi[l, :D] = g["Wi"][l][:, fs]
            wi[l, D] = g["bi"][l][fs]
            wo2[l, :FFC] = g["Wo2"][l][fs, :]
            wo2[l, FFC] = g["bo2"][l] * 0.25
        d.update(wqkkg=wqkkg, wvvg=wvvg, wo=wo, wqg=wqg, wi=wi, wo2=wo2,
                 lnc=lnc, mask=mask,
                 motif=np.concatenate([motif[b], [1.0]]).astype(f32)[:, None],
                 wd=wd_aug, wp=wp_aug)
        in_maps.append({k: (v.astype(np.float32) if k == "lnc"
                            else v.astype(bf16)) for k, v in d.items()})
    return in_maps


_NC_CACHE = {}


def run(inputs, trace=False):
    if "nc" not in _NC_CACHE:
        _NC_CACHE["nc"] = build_program()
    nc = _NC_CACHE["nc"]
    in_maps = prep_inputs(inputs)
    res = run_bass_kernel_spmd(nc, in_maps, core_ids=list(range(N_CORES)),
                               trace=trace)
    out = np.stack([res.results[0]["logits"][:, 0],
                    res.results[TP]["logits"][:, 0]]).astype(np.float32)
    return out, res


def kernel(**inputs) -> np.ndarray:
    out, _ = run(inputs)
    return out



# revision 20
# speedup vs baseline: 1.3627x; 1.3627x over previous
"""Longformer-with-motifs encoder on 8 trn2 NeuronCores.

Sharding: batch(2 groups of 4 cores) x Megatron tensor-parallel(4: 3 heads
each) for attention; FFN + LayerNorms run token-sliced (256 own tokens per
core, full d_ff weights) so the only per-layer collectives are one
ReduceScatter of the attention output (chunked per 512-token span for
overlap) and one small AllGather of the LN2 output.

Device layout: activations feature-major xT [768, S] as SBUF tiles
[128, 6, 1024] bf16.  GEMMs in bf16 (fp32 PSUM accumulation).  Softmax is
max-free (scores are small by construction; masked entries multiplied by
a 0/1 mask post-exp, CLS column handled separately so exp(-1e9)==0
semantics are preserved exactly).  Attention computed transposed
(scoresT[j,i]) so no probability transposes are needed; per-query softmax
sums come from an interleaved ones-column in the PV stationary operand.
LayerNorm (token-sliced): stats via PE ones-matmuls, reciprocal via DVE
reciprocal_approx_fast, rsqrt row broadcast via gpsimd partition_broadcast,
apply via DVE scalar_tensor_tensor + ACT Identity(in*s+b).
"""

import sys

sys.path.insert(0, "/opt/trn_rl_repo")

import numpy as np
import ml_dtypes

import concourse.bacc as bacc
import concourse.bass as bass
import concourse.tile as tile
import concourse.mybir as mybir
from concourse.bass_utils import run_bass_kernel_spmd

BF16 = mybir.dt.bfloat16
F32 = mybir.dt.float32
bf16 = ml_dtypes.bfloat16

B, S, L, H, D, FF, V = 2, 1024, 12, 12, 768, 3072, 50265
DH = D // H
W1 = 256
MAXPOS = 4098
EPS = 1e-5
N_CORES = 8
TP = 4                      # tensor-parallel degree within a group
HC = H // TP                # heads per core = 3
HD = HC * DH                # 192 local head dims
KT = D // 128               # 6 k-tiles over feature dim
FT = FF // 128              # 24 ff tiles
NSP = 2                     # two 512-token spans
SPW = 512
OWN = 256                   # own tokens per core (2 blocks of 128)

# mask tiles: 9 distinct band patterns, keyed by delta = jt*128 - sp*512
_DELTAS = [-256, -128, 0, 128, 256, 384, 512, 640]
_DIDX = {d: i + 1 for i, d in enumerate(_DELTAS)}   # 0 = special key0 tile
NMASK = 9


def _jts(sp):
    return list(range(0, 6)) if sp == 0 else list(range(2, 8))


def _midx(sp, jj):
    jt = _jts(sp)[jj]
    delta = jt * 128 - sp * SPW
    if sp == 0 and jj == 0:
        return 0                  # delta 0 with key-0 masked out
    return _DIDX[delta]


def build_program():
    nc = bacc.Bacc("TRN2", target_bir_lowering=False, debug=False,
                   num_devices=N_CORES)

    def din(name, shape, dt=BF16):
        return nc.dram_tensor(name, shape, dt, kind="ExternalInput").ap()

    x0_d = din("x0", [D, OWN])
    wqkkg_d = din("wqkkg", [L, D + 1, 3 * HD])
    wvvg_d = din("wvvg", [L, D + 1, 2 * HD])
    wo_d = din("wo", [L, HD + 1, D])
    wqg_d = din("wqg", [L, D + 1, HD])
    wi_d = din("wi", [L, D, FF])
    wib_d = din("wib", [L, 128, FT], dt=F32)
    wo2_d = din("wo2", [L, FF + 1, D])
    lnc_d = din("lnc", [L + 1, D, 4], dt=F32)
    mask_d = din("mask", [NMASK, 128, SPW])
    motif_d = din("motif", [415, 1])
    wd_d = din("wd", [1183, D])
    wp_d = din("wp", [D + 1, 2])
    logits_d = nc.dram_tensor("logits", [2, 1], F32, kind="ExternalOutput").ap()

    ACT = mybir.ActivationFunctionType
    ALU = mybir.AluOpType
    GROUPS = [[0, 1, 2, 3], [4, 5, 6, 7]]

    with tile.TileContext(nc) as tc:
        with tc.tile_pool(name="sb1", bufs=1) as p1, \
             tc.tile_pool(name="sb2", bufs=2) as p2, \
             tc.tile_pool(name="sb3", bufs=3) as p3, \
             tc.tile_pool(name="psA", bufs=2, space="PSUM") as psA, \
             tc.tile_pool(name="psS", bufs=2, space="PSUM") as psS, \
             tc.tile_pool(name="psO", bufs=2, space="PSUM") as psO, \
             tc.tile_pool(name="psR", bufs=2, space="PSUM") as psR, \
             tc.tile_pool(name="dram", bufs=2, space="DRAM") as dpool:

            # ---------------- persistent constants ----------------
            ones_row = p1.tile([1, SPW], BF16, tag="ones_row")
            nc.vector.memset(ones_row[:], 1.0)
            ones128 = p1.tile([128, 1], BF16, tag="ones128")
            nc.vector.memset(ones128[:], 1.0)
            ones128f = p1.tile([128, 1], F32, tag="ones128f")
            nc.vector.memset(ones128f[:], 1.0)
            eps_t = p1.tile([1, 1], F32, tag="eps_t")
            nc.vector.memset(eps_t[:], EPS)
            zero_t = p1.tile([128, 1], F32, tag="zero_t")
            nc.vector.memset(zero_t[:], 0.0)
            mask_s = p1.tile([128, NMASK * SPW], BF16, tag="mask_s")
            nc.sync.dma_start(mask_s[:],
                              mask_d[:].rearrange("k p m -> p k m"))

            # activations (persistent tags)
            xb = p1.tile([128, KT, S], BF16, tag="xb")        # bf16 x (GEMM in)
            q_s = p1.tile([64, HC * S], BF16, tag="q_s")
            k_s = p1.tile([64, HC * S], BF16, tag="k_s")
            kg_s = p1.tile([64, HC * S], BF16, tag="kg_s")
            vvg_s = p1.tile([128, 8 * (HC * 65 + HD)], BF16, tag="vvg_s")
            VBLK = HC * 65 + HD                                # 387
            for tt in range(8):
                for h in range(HC):
                    nc.vector.memset(vvg_s[:, tt * VBLK + 65 * h + 64:
                                           tt * VBLK + 65 * h + 65], 1.0)
            att0 = p1.tile([128, S], BF16, tag="att0")         # heads 0,1
            att1 = p1.tile([65, S], BF16, tag="att1")          # head 2 + ones
            nc.vector.memset(att1[64:65, :], 1.0)
            qg_s = p1.tile([64, HC], BF16, tag="qg_s")
            hb = p1.tile([128, FT * OWN], BF16, tag="hb")
            z_loc = p1.tile([128, KT, S], BF16, tag="z_loc")   # attn partial
            z_own = p1.tile([128, KT, OWN], BF16, tag="z_own")
            xq1 = p1.tile([128, KT, OWN], BF16, tag="xq1")     # LN1 out
            xq2 = p1.tile([128, KT, OWN], BF16, tag="xq2", name="xq2")

            def wtile(tag, cols):
                return p1.tile([128, cols], BF16, tag=tag, name=tag)

            # ---------------- helpers ----------------
            def dma_w(t, src, n_k, m, last_rows):
                """load [n_k*128(+last) , m] weight into [128, n_k_tot*m] tile
                (one 3D-AP DMA for the main k-tiles + one for the bias row)"""
                src3 = src[0:n_k * 128, :].rearrange("(k p) m -> p k m",
                                                     p=128, k=n_k)
                nc.sync.dma_start(t[:, 0:n_k * m], src3)
                if last_rows:
                    nc.sync.dma_start(t[0:last_rows, n_k * m:(n_k + 1) * m],
                                      src[n_k * 128:n_k * 128 + last_rows, :])

            def layer_norm_own(z3, lnc_t, c0, dest):
                """z3: [128, KT, OWN] bf16 -> dest [128, KT, OWN] bf16."""
                mp = psR.tile([1, OWN], F32, tag="row")
                mq = psR.tile([1, OWN], F32, tag="row")
                for kt in range(KT):
                    zsl = z3[:, kt, :]
                    zsq = p2.tile([128, OWN], BF16, tag="zsq")
                    nc.scalar.activation(zsq[:], zsl, ACT.Square, bias=zero_t[:])
                    nc.tensor.matmul(mp[:], lhsT=ones128[:], rhs=zsl,
                                     start=(kt == 0), stop=(kt == KT - 1))
                    nc.tensor.matmul(mq[:], lhsT=ones128[:], rhs=zsq[:],
                                     start=(kt == 0), stop=(kt == KT - 1))
                m_s = p1.tile([1, OWN], F32, tag="m_s", name="m_s")
                nc.scalar.activation(m_s[:], mp[:], ACT.Copy, scale=1.0 / D)
                m2 = p1.tile([1, OWN], F32, tag="m2", name="m2")
                nc.scalar.activation(m2[:], m_s[:], ACT.Square, bias=zero_t[0:1, :])
                var = p1.tile([1, OWN], F32, tag="var", name="var")
                nc.vector.scalar_tensor_tensor(
                    var[:], mq[:], 1.0 / D, m2[:], ALU.mult, ALU.subtract)
                std = p1.tile([1, OWN], F32, tag="std", name="std")
                nc.scalar.activation(std[:], var[:], ACT.Sqrt, bias=eps_t[:])
                uf = p1.tile([1, OWN], F32, tag="uf", name="uf")
                nc.vector.reciprocal_approx_fast(uf[:], std[:])
                u_row = p2.tile([1, OWN], BF16, tag="u_row")
                nc.scalar.copy(u_row[:], uf[:])
                w_row = p2.tile([1, OWN], BF16, tag="w_row")
                nc.vector.scalar_tensor_tensor(
                    w_row[:], m_s[:], 1.0, uf[:], ALU.mult, ALU.mult)
                U0 = p2.tile([128, OWN], BF16, tag="U0")
                W0 = p2.tile([128, OWN], BF16, tag="W0")
                nc.gpsimd.partition_broadcast(U0[:], u_row[:])
                nc.gpsimd.partition_broadcast(W0[:], w_row[:])
                for kt in range(KT):
                    s_col = lnc_t[:, 4 * kt + c0: 4 * kt + c0 + 1]
                    b_col = lnc_t[:, 4 * kt + c0 + 1: 4 * kt + c0 + 2]
                    zsl = z3[:, kt, :]
                    t1 = p2.tile([128, OWN], F32, tag="t1")
                    nc.vector.scalar_tensor_tensor(
                        t1[:], zsl, 1.0, U0[:], ALU.mult, ALU.mult)
                    u2 = p2.tile([128, OWN], F32, tag="u2")
                    nc.vector.scalar_tensor_tensor(
                        u2[:], t1[:], 1.0, W0[:], ALU.mult, ALU.subtract)
                    nc.scalar.activation(dest[:, kt, :], u2[:], ACT.Identity,
                                         bias=b_col, scale=s_col)

            def rs_span(sp):
                """ReduceScatter z_loc's span sp over the 4-core group into
                z_own[:, :, sp*128:(sp+1)*128]."""
                bi = dpool.tile([4, 128, KT, 128], BF16, name=f"rs_in{sp}",
                                tag="rs_in")
                for c in range(4):
                    nc.sync.dma_start(
                        bi[c],
                        z_loc[:, :, sp * SPW + c * 128: sp * SPW + (c + 1) * 128])
                bo = dpool.tile([128, KT, 128], BF16, name=f"rs_out{sp}",
                                tag="rs_out")
                nc.gpsimd.collective_compute(
                    "ReduceScatter", ALU.add, replica_groups=GROUPS,
                    ins=[bi[:].opt()], outs=[bo[:].opt()])
                nc.sync.dma_start(
                    z_own[:, :, sp * 128:(sp + 1) * 128], bo[:])

            def ag_x(src):
                """AllGather src [128,KT,OWN] into xb (full tokens)."""
                ai = dpool.tile([128, KT, OWN], BF16, name="ag_in", tag="ag_in")
                nc.sync.dma_start(ai[:], src[:, :, :])
                ao = dpool.tile([4, 128, KT, OWN], BF16, name="ag_out",
                                tag="ag_out")
                nc.gpsimd.collective_compute(
                    "AllGather", ALU.bypass, replica_groups=GROUPS,
                    ins=[ai[:].opt()], outs=[ao[:].opt()])
                for c in range(4):
                    nc.sync.dma_start(xb[:, :, 128 * c: 128 * c + 128],
                                      ao[c, :, :, 0:128])
                    nc.sync.dma_start(xb[:, :, 512 + 128 * c: 512 + 128 * c + 128],
                                      ao[c, :, :, 128:256])

            # ---------------- embeddings ----------------
            for kt in range(KT):
                nc.sync.dma_start(z_own[:, kt, :],
                                  x0_d[kt * 128:(kt + 1) * 128, :])
            lnc_e = p2.tile([128, 4 * KT], F32, tag="lnc")
            nc.sync.dma_start(lnc_e[:], lnc_d[L].rearrange(
                "(k p) m -> p k m", p=128, k=KT))
            layer_norm_own(z_own, lnc_e, 0, xq2)
            ag_x(xq2)

            # ---------------- layers ----------------
            for l in range(L):
                wqkkg = wtile("wqkkg", 7 * 3 * HD)
                dma_w(wqkkg, wqkkg_d[l], KT, 3 * HD, 1)
                wvvg = wtile("wvvg", 7 * 2 * HD)
                dma_w(wvvg, wvvg_d[l], KT, 2 * HD, 1)
                wqg = wtile("wqg", 7 * HD)
                dma_w(wqg, wqg_d[l], KT, HD, 1)
                wo_s = wtile("wo_s", 2 * D)
                nc.sync.dma_start(wo_s[:, 0:D], wo_d[l, 0:128, :])
                nc.sync.dma_start(wo_s[0:65, D:2 * D], wo_d[l, 128:193, :])
                lnc_t = p2.tile([128, 4 * KT], F32, tag="lnc")
                nc.sync.dma_start(lnc_t[:], lnc_d[l].rearrange(
                    "(k p) m -> p k m", p=128, k=KT))
                wi_s = wtile("wi_s", KT * FF)
                dma_w(wi_s, wi_d[l], KT, FF, 0)
                wib_s = p2.tile([128, FT], F32, tag="wib")
                nc.sync.dma_start(wib_s[:], wib_d[l])
                wo2_s = wtile("wo2_s", 25 * D)
                dma_w(wo2_s, wo2_d[l], FT, D, 1)

                # ---- qkv/kg projections: out[64m, tok] ----
                for sp in range(NSP):
                    for mt in range(5):
                        mw = 128 if mt < 4 else 64
                        ps = psA.tile([128, SPW], F32, tag="psA")
                        for kt in range(KT + 1):
                            kk = 128 if kt < KT else 1
                            lhsT = wqkkg[0:kk, kt * 3 * HD + mt * 128:
                                         kt * 3 * HD + mt * 128 + mw]
                            rhs = (xb[:, kt, sp * SPW:(sp + 1) * SPW]
                                   if kt < KT else
                                   ones_row[0:1, 0:SPW])
                            nc.tensor.matmul(ps[0:mw, :], lhsT=lhsT, rhs=rhs,
                                             start=(kt == 0), stop=(kt == KT))
                        for sub in range(2 if mt < 4 else 1):
                            m = 2 * mt + sub
                            kind, h = m // 3, m % 3
                            dest = (q_s, k_s, kg_s)[kind]
                            nc.vector.tensor_copy(
                                dest[0:64, h * S + sp * SPW: h * S + (sp + 1) * SPW],
                                ps[64 * sub:64 * sub + 64, :])

                # ---- v/vg projections: out[tok, dh] ----
                for tt in range(8):
                    ps = psA.tile([128, 2 * HD], F32, tag="psA")
                    for kt in range(KT + 1):
                        kk = 128 if kt < KT else 1
                        lhsT = (xb[:, kt, tt * 128:(tt + 1) * 128]
                                if kt < KT else ones_row[0:1, 0:128])
                        rhs = wvvg[0:kk, kt * 2 * HD:(kt + 1) * 2 * HD]
                        nc.tensor.matmul(ps[:], lhsT=lhsT, rhs=rhs,
                                         start=(kt == 0), stop=(kt == KT))
                    base = tt * VBLK
                    for h in range(HC):
                        nc.vector.tensor_copy(
                            vvg_s[:, base + 65 * h: base + 65 * h + 64],
                            ps[:, 64 * h:64 * h + 64])
                    nc.vector.tensor_copy(
                        vvg_s[:, base + 65 * HC: base + 65 * HC + HD],
                        ps[:, HD:2 * HD])

                # ---- global query projection qgT [192, 1] ----
                for mt in range(2):
                    mw = 128 if mt == 0 else 64
                    ps = psR.tile([128, 1], F32, tag="row")
                    for kt in range(KT + 1):
                        kk = 128 if kt < KT else 1
                        lhsT = wqg[0:kk, kt * HD + mt * 128: kt * HD + mt * 128 + mw]
                        rhs = (xb[:, kt, 0:1] if kt < KT
                               else ones_row[0:1, 0:1])
                        nc.tensor.matmul(ps[0:mw, :], lhsT=lhsT, rhs=rhs,
                                         start=(kt == 0), stop=(kt == KT))
                    for sub in range(2 if mt == 0 else 1):
                        h = 2 * mt + sub
                        nc.vector.tensor_copy(qg_s[0:64, h:h + 1],
                                              ps[64 * sub:64 * sub + 64, :])

                # ---- attention ----
                for h in range(HC):
                    # global attention for this head -> og [65,1]
                    sg = psS.tile([128, 8], F32, tag="sc")
                    for jt in range(8):
                        nc.tensor.matmul(
                            sg[:, jt:jt + 1],
                            lhsT=kg_s[0:64, h * S + jt * 128: h * S + (jt + 1) * 128],
                            rhs=qg_s[0:64, h:h + 1], start=True, stop=True)
                    esg = p2.tile([128, 8], BF16, tag="esg")
                    acc = p2.tile([128, 1], F32, tag="acc_sg")
                    nc.scalar.activation(esg[:], sg[:], ACT.Exp, bias=zero_t[:], accum_out=acc[:])
                    og = psR.tile([65, 1], F32, tag="row")
                    nc.tensor.matmul(og[64:65, :], lhsT=ones128f[:], rhs=acc[:],
                                     start=True, stop=True)
                    for jt in range(8):
                        nc.tensor.matmul(
                            og[0:64, :],
                            lhsT=vvg_s[:, jt * VBLK + 65 * HC + 64 * h:
                                       jt * VBLK + 65 * HC + 64 * h + 64],
                            rhs=esg[:, jt:jt + 1],
                            start=(jt == 0), stop=(jt == 7))
                    for sp in range(NSP):
                        outT = psO.tile([65, SPW], F32, tag="outT")
                        jts = _jts(sp)
                        for jj, jt in enumerate(jts):
                            sc = psS.tile([128, SPW], F32, tag="sc")
                            nc.tensor.matmul(
                                sc[:],
                                lhsT=k_s[0:64, h * S + jt * 128: h * S + (jt + 1) * 128],
                                rhs=q_s[0:64, h * S + sp * SPW: h * S + (sp + 1) * SPW],
                                start=True, stop=True)
                            ex = p2.tile([128, SPW], BF16, tag="ex")
                            nc.scalar.activation(ex[:], sc[:], ACT.Exp, bias=zero_t[:])
                            exm = p2.tile([128, SPW], BF16, tag="exm")
                            midx = _midx(sp, jj)
                            nc.vector.scalar_tensor_tensor(
                                exm[:], ex[:], 1.0,
                                mask_s[:, midx * SPW:(midx + 1) * SPW],
                                ALU.mult, ALU.mult)
                            nc.tensor.matmul(
                                outT[:],
                                lhsT=vvg_s[:, jt * VBLK + 65 * h: jt * VBLK + 65 * h + 65],
                                rhs=exm[:], start=(jj == 0), stop=False)
                        # CLS column (key 0) for all queries
                        csc = psR.tile([1, SPW], F32, tag="row")
                        nc.tensor.matmul(
                            csc[:], lhsT=k_s[0:64, h * S: h * S + 1],
                            rhs=q_s[0:64, h * S + sp * SPW: h * S + (sp + 1) * SPW],
                            start=True, stop=True)
                        cex = p2.tile([1, SPW], BF16, tag="cex")
                        nc.scalar.activation(cex[:], csc[:], ACT.Exp, bias=zero_t[0:1, :])
                        nc.tensor.matmul(outT[:],
                                         lhsT=vvg_s[0:1, 65 * h: 65 * h + 65],
                                         rhs=cex[:], start=False, stop=True)
                        if sp == 0:
                            # overwrite CLS token output with global attention
                            nc.vector.tensor_copy(outT[0:65, 0:1], og[0:65, :])
                        # normalize by the sums row and store
                        sums = p1.tile([1, SPW], F32, tag="sums", name="sums")
                        nc.scalar.copy(sums[:], outT[64:65, :])
                        rr = p1.tile([1, SPW], F32, tag="rr", name="rr")
                        nc.vector.reciprocal_approx_fast(rr[:], sums[:])
                        rrb = p2.tile([1, SPW], BF16, tag="rrb")
                        nc.scalar.copy(rrb[:], rr[:])
                        rb = p1.tile([64, SPW], BF16, tag="rb", name="rb")
                        nc.gpsimd.partition_broadcast(rb[:], rrb[:])
                        dest = (att0[64 * h:64 * h + 64,
                                     sp * SPW:(sp + 1) * SPW] if h < 2 else
                                att1[0:64, sp * SPW:(sp + 1) * SPW])
                        nc.vector.scalar_tensor_tensor(
                            dest, outT[0:64, :], 1.0, rb[:], ALU.mult, ALU.mult)

                # ---- output projection + residual; RS per span ----
                for sp in range(NSP):
                    for mt in range(KT):
                        ps = psA.tile([128, SPW], F32, tag="psA")
                        nc.tensor.matmul(
                            ps[:], lhsT=wo_s[:, mt * 128:(mt + 1) * 128],
                            rhs=att0[:, sp * SPW:(sp + 1) * SPW],
                            start=True, stop=False)
                        nc.tensor.matmul(
                            ps[:], lhsT=wo_s[0:65, D + mt * 128: D + (mt + 1) * 128],
                            rhs=att1[:, sp * SPW:(sp + 1) * SPW],
                            start=False, stop=True)
                        nc.vector.scalar_tensor_tensor(
                            z_loc[:, mt, sp * SPW:(sp + 1) * SPW],
                            xb[:, mt, sp * SPW:(sp + 1) * SPW],
                            0.25, ps[:], ALU.mult, ALU.add)
                    rs_span(sp)

                layer_norm_own(z_own, lnc_t, 0, xq1)

                # ---- FFN (token-sliced, full d_ff) ----
                for mtf in range(FT):
                    ps = psA.tile([128, OWN], F32, tag="psA")
                    for kt in range(KT):
                        lhsT = wi_s[:, kt * FF + mtf * 128:
                                    kt * FF + mtf * 128 + 128]
                        nc.tensor.matmul(ps[:], lhsT=lhsT, rhs=xq1[:, kt, :],
                                         start=(kt == 0), stop=(kt == KT - 1))
                    nc.scalar.activation(
                        hb[:, mtf * OWN:(mtf + 1) * OWN],
                        ps[:], ACT.Gelu, bias=wib_s[:, mtf:mtf + 1])
                z2_own = p1.tile([128, KT, OWN], BF16, tag="z2_own",
                                 name="z2_own")
                for mt in range(KT):
                    ps = psA.tile([128, OWN], F32, tag="psA")
                    for ktf in range(FT + 1):
                        kk = 128 if ktf < FT else 1
                        lhsT = wo2_s[0:kk, ktf * D + mt * 128:
                                     ktf * D + (mt + 1) * 128]
                        rhs = (hb[:, ktf * OWN:(ktf + 1) * OWN]
                               if ktf < FT else ones_row[0:1, 0:OWN])
                        nc.tensor.matmul(ps[:], lhsT=lhsT, rhs=rhs,
                                         start=(ktf == 0), stop=(ktf == FT))
                    nc.vector.scalar_tensor_tensor(
                        z2_own[:, mt, :], xq1[:, mt, :],
                        1.0, ps[:], ALU.mult, ALU.add)
                layer_norm_own(z2_own, lnc_t, 2, xq2)
                ag_x(xq2)

            # ---------------- classification head ----------------
            wd_s = wi_s
            for kt in range(9):
                nc.sync.dma_start(wd_s[:, kt * D:(kt + 1) * D],
                                  wd_d[kt * 128:(kt + 1) * 128, :])
            nc.sync.dma_start(wd_s[0:31, 9 * D:10 * D], wd_d[9 * 128:1183, :])
            wp_s = p1.tile([128, 14], BF16, tag="wp_s")
            for kt in range(6):
                nc.sync.dma_start(wp_s[:, 2 * kt:2 * kt + 2],
                                  wp_d[kt * 128:(kt + 1) * 128, :])
            nc.sync.dma_start(wp_s[0:1, 12:14], wp_d[768:769, :])
            mot_s = p1.tile([128, 4], BF16, tag="mot_s")
            for c in range(4):
                sz = 128 if c < 3 else 31
                nc.sync.dma_start(mot_s[0:sz, c:c + 1],
                                  motif_d[128 * c:128 * c + sz, :])

            ty = p1.tile([128, KT], BF16, tag="ty")
            for mt in range(KT):
                ps = psR.tile([128, 1], F32, tag="row")
                for kt in range(10):
                    kk = 128 if kt < 9 else 31
                    lhsT = wd_s[0:kk, kt * D + mt * 128: kt * D + (mt + 1) * 128]
                    rhs = (xb[:, kt, 0:1] if kt < KT
                           else mot_s[0:kk, kt - KT: kt - KT + 1])
                    nc.tensor.matmul(ps[:], lhsT=lhsT, rhs=rhs,
                                     start=(kt == 0), stop=(kt == 9))
                nc.scalar.activation(ty[:, mt:mt + 1], ps[:], ACT.Tanh, bias=zero_t[:])
            lg_ps = psR.tile([2, 1], F32, tag="row")
            for kt in range(7):
                kk = 128 if kt < 6 else 1
                lhsT = wp_s[0:kk, 2 * kt:2 * kt + 2]
                rhs = ty[:, kt:kt + 1] if kt < 6 else ones_row[0:1, 0:1]
                nc.tensor.matmul(lg_ps[:], lhsT=lhsT, rhs=rhs,
                                 start=(kt == 0), stop=(kt == 6))
            lg_s = p1.tile([2, 1], F32, tag="lg_s")
            nc.vector.tensor_copy(lg_s[:], lg_ps[:])
            nc.sync.dma_start(logits_d[:], lg_s[:])

    nc.compile()
    return nc


def prep_inputs(inputs):
    """host-side sharding: returns in_maps for the 8 cores"""
    f32 = np.float32
    ids = np.asarray(inputs["input_ids"])
    motif = np.asarray(inputs["motif_dist"], f32)
    emb_word = np.asarray(inputs["emb_word"], f32)
    emb_pos = np.asarray(inputs["emb_pos"], f32)
    emb_type = np.asarray(inputs["emb_type"], f32)
    g = {k: np.asarray(inputs[k], f32) for k in
         ("Wq", "bq", "Wk", "bk", "Wv", "bv", "Wqg", "bqg", "Wkg", "bkg",
          "Wvg", "bvg", "Wo", "bo", "ln1_s", "ln1_b", "Wi", "bi", "Wo2",
          "bo2", "ln2_s", "ln2_b", "emb_ln_s", "emb_ln_b",
          "head_Wd", "head_bd", "head_Wp", "head_bp")}
    scale = 1.0 / np.sqrt(DH)

    # masks [NMASK, 128, 512]: tile 0 = delta 0 with key0 masked; then bands
    mask = np.zeros((NMASK, 128, SPW), f32)
    p = np.arange(128)[:, None]
    q = np.arange(SPW)[None, :]
    mask[0] = ((np.abs(p - q) <= W1) & (p != 0)).astype(f32)
    for d, i in _DIDX.items():
        mask[i] = (np.abs(d + p - q) <= W1).astype(f32)

    # lnc [13, 768, 4] : (s1, b1, s2, b2) per layer; [12] = embedding LN
    lnc = np.zeros((L + 1, D, 4), f32)
    for l in range(L):
        lnc[l, :, 0] = g["ln1_s"][l]
        lnc[l, :, 1] = g["ln1_b"][l]
        lnc[l, :, 2] = g["ln2_s"][l]
        lnc[l, :, 3] = g["ln2_b"][l]
    lnc[L, :, 0] = g["emb_ln_s"]
    lnc[L, :, 1] = g["emb_ln_b"]

    wd_aug = np.concatenate([g["head_Wd"], g["head_bd"][None, :]], 0)  # [1183,768]
    wp_aug = np.concatenate([g["head_Wp"], g["head_bp"][None, :]], 0)  # [769,2]

    # full-FFN weights (shared by all cores)
    wi = np.ascontiguousarray(g["Wi"])                       # [L, D, FF]
    wib = np.transpose(g["bi"].reshape(L, FT, 128), (0, 2, 1)).copy()
    wo2 = np.zeros((L, FF + 1, D), f32)
    wo2[:, :FF] = g["Wo2"]
    wo2[:, FF] = g["bo2"]

    in_maps = []
    for core in range(N_CORES):
        b, r = core // TP, core % TP
        hs = slice(HD * r, HD * (r + 1))
        x0 = (emb_word[ids[b]] + emb_pos[2:2 + S] + emb_type[0]).T  # [D, S]
        x0_own = np.concatenate(
            [x0[:, 128 * r:128 * r + 128],
             x0[:, 512 + 128 * r:512 + 128 * r + 128]], axis=1)     # [D, 256]
        d = {"x0": np.ascontiguousarray(x0_own)}
        wqkkg = np.zeros((L, D + 1, 3 * HD), f32)
        wvvg = np.zeros((L, D + 1, 2 * HD), f32)
        wo = np.zeros((L, HD + 1, D), f32)
        wqg = np.zeros((L, D + 1, HD), f32)
        for l in range(L):
            wqkkg[l, :D] = np.concatenate(
                [g["Wq"][l][:, hs] * scale, g["Wk"][l][:, hs],
                 g["Wkg"][l][:, hs]], 1)
            wqkkg[l, D] = np.concatenate(
                [g["bq"][l][hs] * scale, g["bk"][l][hs], g["bkg"][l][hs]])
            wvvg[l, :D] = np.concatenate(
                [g["Wv"][l][:, hs], g["Wvg"][l][:, hs]], 1)
            wvvg[l, D] = np.concatenate([g["bv"][l][hs], g["bvg"][l][hs]])
            wo[l, :HD] = g["Wo"][l][hs, :]
            wo[l, HD] = g["bo"][l] * 0.25
            wqg[l, :D] = g["Wqg"][l][:, hs] * scale
            wqg[l, D] = g["bqg"][l][hs] * scale
        d.update(wqkkg=wqkkg, wvvg=wvvg, wo=wo, wqg=wqg, wi=wi, wib=wib,
                 wo2=wo2, lnc=lnc, mask=mask,
                 motif=np.concatenate([motif[b], [1.0]]).astype(f32)[:, None],
                 wd=wd_aug, wp=wp_aug)
        in_maps.append({k: (v.astype(np.float32) if k in ("lnc", "wib")
                            else v.astype(bf16)) for k, v in d.items()})
    return in_maps


_NC_CACHE = {}


def run(inputs, trace=False):
    if "nc" not in _NC_CACHE:
        _NC_CACHE["nc"] = build_program()
    nc = _NC_CACHE["nc"]
    in_maps = prep_inputs(inputs)
    res = run_bass_kernel_spmd(nc, in_maps, core_ids=list(range(N_CORES)),
                               trace=trace)
    out = np.stack([res.results[0]["logits"][:, 0],
                    res.results[TP]["logits"][:, 0]]).astype(np.float32)
    return out, res


def kernel(**inputs) -> np.ndarray:
    out, _ = run(inputs)
    return out
